# revision 22
# baseline (speedup 1.0000x reference)
"""Trainium2 Bass kernel for ANHP multi-head self-attention.

Problem: out[b] = softmax(exp((x Wq + bq)(x Wk + bk)^T / sqrt(dh)) + causal_soft_mask) (x Wv + bv)
Shapes: B=8, S=1024, FEAT=HID=1024, H=8 heads, DH=128.

Sharding: pure data parallel — batch element b -> NeuronCore b. No collectives.

Per-core dataflow (all layouts chosen so no on-device transposes are needed):
  - host passes xT = x.T packed partition-major (fat DMA descriptors),
  - qT/kT are produced as [dh, S] per head (projection with W as lhsT),
  - scores are computed directly transposed: scT[sk, sq] = k_sk . q_sq / sqrt(dh),
  - softmax numerator E = exp(exp(scT)) with the causal mask applied
    multiplicatively (reference's additive -128 soft mask underflows exp to 0),
  - blocks with sk > sq (fully masked) are skipped entirely; E is stored as a
    packed trapezoid per head, split into lo/hi tiles for finer ctx overlap,
  - the softmax denominator comes for free as a ones-column appended to v
    (column 128 of each 129-wide head block of v~),
  - ctx[sq, dh] = sum_sk E[sk, sq] * v~[sk, :] accumulated on PSUM, then
    normalized by the per-partition reciprocal of the denominator column,
  - output is written head-major partition-major; host reassembles [S, HID].
"""

import numpy as np
import ml_dtypes

import concourse.bass as bass
import concourse.bacc as bacc
import concourse.mybir as mybir
import concourse.tile as tile
from concourse.bass_utils import run_bass_kernel_spmd

B, S, FEAT, HID, H, DH = 8, 1024, 1024, 1024, 8, 128
NF = FEAT // 128  # f-blocks
NS = S // 128  # s-blocks
EOFF = [0]
for _bi in range(1, 9):
    EOFF.append(EOFF[-1] + 1024 - 128 * (_bi - 1))  # EOFF[8] = 4608
SCALE = 1.0 / float(np.sqrt(DH))
F32 = mybir.dt.float32
BF16 = mybir.dt.bfloat16
BF_NP = ml_dtypes.bfloat16
EXP = mybir.ActivationFunctionType.Exp

_CACHED_NC = None


def build_nc():
    nc = bacc.Bacc()
    # all inputs packed partition-major on host for fat DMA descriptors
    xt_d = nc.declare_dram_parameter("xt", [128, NF, S], BF16, isOutput=False)
    wqk_d = nc.declare_dram_parameter("wqk", [H, 128, 2, NF, 128], BF16, isOutput=False)
    wv_d = nc.declare_dram_parameter("wv", [128, NF, HID], BF16, isOutput=False)
    # constf columns: 0:8 bq^T, 8:16 bk^T, 16:1040 bv broadcast
    constf_d = nc.declare_dram_parameter("constf", [128, 1040], F32, isOutput=False)
    trim_d = nc.declare_dram_parameter("trim", [128, 128], BF16, isOutput=False)
    out_d = nc.declare_dram_parameter("out", [H, 128, NS, 128], F32, isOutput=True)

    with tile.TileContext(nc) as tc:
        with (
            tc.tile_pool(name="const", bufs=1) as constp,
            tc.tile_pool(name="xt", bufs=1) as xtp,
            tc.tile_pool(name="wv", bufs=1) as wvp,
            tc.tile_pool(name="wcol", bufs=3) as wcolp,
            tc.tile_pool(name="qk", bufs=1) as qkp,
            tc.tile_pool(name="vt", bufs=1) as vp,
            tc.tile_pool(name="score", bufs=4) as scorep,
            tc.tile_pool(name="ebuf", bufs=3) as ep,
            tc.tile_pool(name="outt", bufs=2) as outp,
            tc.tile_pool(name="rcp", bufs=4) as rcpp,
            tc.tile_pool(name="psA", bufs=3, space=bass.MemorySpace.PSUM) as psA,
            tc.tile_pool(name="psS", bufs=2, space=bass.MemorySpace.PSUM) as psS,
            tc.tile_pool(name="psC", bufs=3, space=bass.MemorySpace.PSUM) as psC,
        ):
            # constants (gpsimd queue, issued first)
            constf = constp.tile([128, 1040], F32, name="constf", tag="constf")
            nc.gpsimd.dma_start(constf[:], constf_d[:])
            trim = constp.tile([128, 128], BF16, name="trim", tag="trim")
            nc.gpsimd.dma_start(trim[:], trim_d[:])
            bqt = constf[:, 0:8]
            bkt = constf[:, 8:16]
            bvb = constf[:, 16:1040]

            # PE warmup: dummy matmuls with no DMA deps bridge the DMA ramp and
            # flip the HAM clock gate to 2.4GHz before real matmuls arrive.
            wu_a = constp.tile([128, 128], BF16, name="wu_a", tag="wu_a")
            nc.vector.memset(wu_a[:], 0.0)
            wu_b = constp.tile([128, 512], BF16, name="wu_b", tag="wu_b")
            nc.vector.memset(wu_b[:], 0.0)
            for _ in range(10):
                wups = psS.tile([128, 512], F32, name="wups", tag="psS")
                nc.tensor.matmul(wups[:], wu_a[:], wu_b[:], start=True, stop=True)

            wc_of = {}

            def load_wcol(h, eng=None):
                eng = eng or nc.scalar
                wqk = wcolp.tile([128, 2, NF, 128], BF16, name="wqk", tag="wqk")
                eng.dma_start(wqk[:], wqk_d[h])
                wc_of[h] = wqk

            # first head's weights on the scalar queue; xT thirds across
            # the three DMA queues in consumption order
            load_wcol(0, nc.scalar)
            xtq = [
                xtp.tile([128, 1, S], BF16, name="xtf0", tag="xtf0"),
                xtp.tile([128, 2, S], BF16, name="xtq0", tag="xtq0"),
                xtp.tile([128, 3, S], BF16, name="xtq1", tag="xtq1"),
                xtp.tile([128, 2, S], BF16, name="xtq2", tag="xtq2"),
            ]
            nc.sync.dma_start(xtq[0][:], xt_d[:, 0:1, :])
            nc.sync.dma_start(xtq[1][:], xt_d[:, 1:3, :])
            nc.scalar.dma_start(xtq[2][:], xt_d[:, 3:6, :])
            nc.gpsimd.dma_start(xtq[3][:], xt_d[:, 6:8, :])

            _XT = [(0, 0), (1, 0), (1, 1), (2, 0), (2, 1), (2, 2), (3, 0), (3, 1)]

            def xts(fb):
                t, j = _XT[fb]
                return xtq[t][:, j, :]

            # big persistent tensors
            qT = qkp.tile([128, H, S], BF16, name="qT", tag="qT")  # [dh, head, s]
            kT = qkp.tile([128, H, S], BF16, name="kT", tag="kT")
            vv = vp.tile([128, NS, H * 129], BF16, name="vv", tag="vv")
            wv_sb = wvp.tile([128, NF, HID], BF16, name="wv", tag="wv")
            nc.gpsimd.dma_start(wv_sb[:], wv_d[:])

            # ones columns of v~ (denominator trick)
            ones_ap = vv.rearrange("p si (h d) -> p si h d", d=129)[:, :, :, 128:129]
            nc.vector.memset(ones_ap, 1.0)

            def proj_qk(h):
                # f-outer, two PSUM groups at a time (q then k) so a PSUM bank
                # stays free for ctx recycling.
                wqk = wc_of.pop(h)
                for qk, bt, dest in ((0, bqt, qT), (1, bkt, kT)):
                    pss = [
                        psA.tile([128, 512], F32, name="psA", tag="psA")
                        for _ in range(2)
                    ]
                    for fb in range(NF):
                        for c in range(2):
                            nc.tensor.matmul(
                                pss[c][:],
                                wqk[:, qk, fb, :],
                                xts(fb)[:, c * 512 : (c + 1) * 512],
                                start=(fb == 0),
                                stop=(fb == NF - 1),
                            )
                    for c in range(2):
                        nc.vector.tensor_scalar_add(
                            dest[:, h, c * 512 : (c + 1) * 512],
                            pss[c][:],
                            bt[:, h : h + 1],
                        )

            def proj_v():
                for si in range(NS):
                    for c in range(2):
                        ps = psA.tile([128, 512], F32, name="psA", tag="psA")
                        for fb in range(NF):
                            nc.tensor.matmul(
                                ps[:],
                                xts(fb)[:, si * 128 : (si + 1) * 128],
                                wv_sb[:, fb, c * 512 : (c + 1) * 512],
                                start=(fb == 0),
                                stop=(fb == NF - 1),
                            )
                        dst = vv[:, si, :].rearrange("p (h d) -> p h d", d=129)[
                            :, 4 * c : 4 * c + 4, 0:128
                        ]
                        nc.vector.tensor_add(
                            dst,
                            ps[:].rearrange("p (h d) -> p h d", d=128),
                            bvb[:, c * 512 : (c + 1) * 512].rearrange(
                                "p (h d) -> p h d", d=128
                            ),
                        )

            def eslice(E, bi, a, b):
                # packed-trapezoid slice of strip bi, local cols [a, b)
                t = E[0] if bi < 4 else E[1]
                o = EOFF[bi] - (0 if bi < 4 else EOFF[4])
                return t[:, o + a : o + b]

            def attn_scores(h, E):
                # E packed trapezoid: strip bi holds sq in [bi*128, S) at
                # columns [EOFF[bi], EOFF[bi+1])
                for bi in range(NS):
                    st = scorep.tile([128, S], F32, name="st", tag="st")
                    lo = bi * 128
                    for c0 in range(lo, S, 512):
                        n = min(512, S - c0)
                        ps = psS.tile([128, 512], F32, name="psS", tag="psS")
                        nc.tensor.matmul(
                            ps[:, 0:n],
                            kT[:, h, lo : lo + 128],
                            qT[:, h, c0 : c0 + n],
                            start=True,
                            stop=True,
                        )
                        nc.scalar.activation(
                            st[:, c0 : c0 + n], ps[:, 0:n], EXP, scale=SCALE
                        )
                    eb = eslice(E, bi, 0, S - lo)
                    nc.scalar.activation(eb, st[:, lo:S], EXP)
                    ebd = eslice(E, bi, 0, 128)
                    meng = nc.gpsimd if h >= H - 2 else nc.vector
                    meng.tensor_mul(ebd, ebd, trim[:])

            def ctx(h, E):
                oh = outp.tile([128, NS, 128], F32, name="oh", tag="oh")
                for bj in range(NS):
                    ps = psC.tile([128, 129], F32, name="psC", tag="psC")
                    for bi in range(bj + 1):
                        nc.tensor.matmul(
                            ps[:],
                            eslice(E, bi, (bj - bi) * 128, (bj - bi) * 128 + 128),
                            vv[:, bi, h * 129 : h * 129 + 129],
                            start=(bi == 0),
                            stop=(bi == bj),
                        )
                    rc = rcpp.tile([128, 1], F32, name="rc", tag="rc")
                    nc.vector.reciprocal(rc[:], ps[:, 128:129])
                    nc.vector.tensor_scalar_mul(oh[:, bj, :], ps[:, 0:128], rc[:])
                # head-major partition-major output, split per bj-pair so the
                # final transfer after the last normalize is small
                for j0 in range(0, NS, 2):
                    nc.sync.dma_start(out_d[h][:, j0 : j0 + 2, :], oh[:, j0 : j0 + 2, :])

            E_of = {}

            def head_front(h):
                if h + 1 < H:
                    load_wcol(h + 1)
                proj_qk(h)
                elo = ep.tile([128, EOFF[4]], BF16, name="Elo", tag="Elo")
                ehi = ep.tile([128, EOFF[NS] - EOFF[4]], BF16, name="Ehi", tag="Ehi")
                E_of[h] = (elo, ehi)
                attn_scores(h, E_of[h])

            head_front(0)
            head_front(1)
            head_front(2)
            proj_v()
            ctx(0, E_of.pop(0))
            for h in range(3, H):
                head_front(h)
                ctx(h - 2, E_of.pop(h - 2))
            ctx(H - 2, E_of.pop(H - 2))
            ctx(H - 1, E_of.pop(H - 1))

    nc.finalize()
    return nc


def _get_nc():
    global _CACHED_NC
    if _CACHED_NC is None:
        _CACHED_NC = build_nc()
    return _CACHED_NC


def _prep_shared(Wq, bq, Wk, bk, Wv, bv):
    def reorder(w):
        # [f, n] -> [h, f_in_blk(partition), f_blk, c]
        return w.reshape(NF, 128, H, 128).transpose(2, 1, 0, 3)

    wqk = np.ascontiguousarray(
        np.stack([reorder(Wq), reorder(Wk)], axis=2)
    ).astype(BF_NP)  # [H, 128, 2, NF, 128]
    wvh = np.ascontiguousarray(
        Wv.reshape(NF, 128, HID).transpose(1, 0, 2)
    ).astype(BF_NP)  # [128, NF, HID]
    constf = np.empty((128, 1040), np.float32)
    constf[:, 0:8] = bq.reshape(H, 128).T
    constf[:, 8:16] = bk.reshape(H, 128).T
    constf[:, 16:1040] = np.broadcast_to(bv, (128, HID))
    return dict(
        wqk=wqk,
        wv=wvh,
        constf=constf,
        trim=np.triu(np.ones((128, 128), np.float32)).astype(BF_NP),
    )


def _prep_xt(x):
    # [S, F] -> xT [F, S] -> [128(p), NF, S] partition-major, bf16
    return np.ascontiguousarray(x.T.reshape(NF, 128, S).transpose(1, 0, 2)).astype(
        BF_NP
    )


def _unpack_out(arr):
    # [H, 128, NS, 128] -> [S, HID]
    return np.ascontiguousarray(arr.transpose(2, 1, 0, 3).reshape(S, HID))


def kernel(queries, Wq, bq, Wk, bk, Wv, bv):
    queries = np.asarray(queries, np.float32)
    shared = _prep_shared(
        np.asarray(Wq, np.float32),
        np.asarray(bq, np.float32),
        np.asarray(Wk, np.float32),
        np.asarray(bk, np.float32),
        np.asarray(Wv, np.float32),
        np.asarray(bv, np.float32),
    )
    in_maps = [dict(xt=_prep_xt(queries[b]), **shared) for b in range(B)]
    nc = _get_nc()
    res = run_bass_kernel_spmd(nc, in_maps, core_ids=list(range(B)))
    return np.stack([_unpack_out(res.results[b]["out"]) for b in range(B)], axis=0)


if __name__ == "__main__":
    rng = np.random.default_rng(0)
    q = rng.standard_normal((B, S, FEAT), dtype=np.float32)
    Wq = (rng.standard_normal((FEAT, HID), dtype=np.float32) * 0.02).astype(np.float32)
    Wk = (rng.standard_normal((FEAT, HID), dtype=np.float32) * 0.02).astype(np.float32)
    Wv = (rng.standard_normal((FEAT, HID), dtype=np.float32) * 0.02).astype(np.float32)
    z = np.zeros(HID, np.float32)
    out = kernel(queries=q, Wq=Wq, bq=z, Wk=Wk, bk=z, Wv=Wv, bv=z)
    print(out.shape, out.dtype)


# revision 23
# speedup vs baseline: 1.1982x; 1.1982x over previous
"""Trainium2 Bass kernel for ANHP multi-head self-attention.

Problem: out[b] = softmax(exp((x Wq + bq)(x Wk + bk)^T / sqrt(dh)) + causal_soft_mask) (x Wv + bv)
Shapes: B=8, S=1024, FEAT=HID=1024, H=8 heads, DH=128.

Sharding: pure data parallel — batch element b -> NeuronCore b. No collectives.

Per-core dataflow (all layouts chosen so no on-device transposes are needed):
  - host passes xT = x.T packed partition-major (fat DMA descriptors),
  - qT/kT are produced as [dh, S] per head (projection with W as lhsT),
  - scores are computed directly transposed: scT[sk, sq] = k_sk . q_sq / sqrt(dh),
  - softmax numerator E = exp(exp(scT)) with the causal mask applied
    multiplicatively (reference's additive -128 soft mask underflows exp to 0),
  - blocks with sk > sq (fully masked) are skipped entirely; E is stored as a
    packed trapezoid per head, split into lo/hi tiles for finer ctx overlap,
  - the softmax denominator comes for free as a ones-column appended to v
    (column 128 of each 129-wide head block of v~),
  - ctx[sq, dh] = sum_sk E[sk, sq] * v~[sk, :] accumulated on PSUM, then
    normalized by the per-partition reciprocal of the denominator column,
  - output is written head-major partition-major; host reassembles [S, HID].
"""

import numpy as np
import ml_dtypes

import concourse.bass as bass
import concourse.bacc as bacc
import concourse.mybir as mybir
import concourse.tile as tile
from concourse.bass_utils import run_bass_kernel_spmd

B, S, FEAT, HID, H, DH = 8, 1024, 1024, 1024, 8, 128
NF = FEAT // 128  # f-blocks
NS = S // 128  # s-blocks
EOFF = [0]
for _bi in range(1, 9):
    EOFF.append(EOFF[-1] + 1024 - 128 * (_bi - 1))  # EOFF[8] = 4608
SCALE = 1.0 / float(np.sqrt(DH))
F32 = mybir.dt.float32
BF16 = mybir.dt.bfloat16
BF_NP = ml_dtypes.bfloat16
EXP = mybir.ActivationFunctionType.Exp

_CACHED_NC = None


def build_nc():
    nc = bacc.Bacc()
    # all inputs packed partition-major on host for fat DMA descriptors
    xt_d = nc.declare_dram_parameter("xt", [128, NF, S], BF16, isOutput=False)
    wqk_d = nc.declare_dram_parameter("wqk", [H, 128, 2, NF, 128], BF16, isOutput=False)
    wv_d = nc.declare_dram_parameter("wv", [128, NF, HID], BF16, isOutput=False)
    # constf columns: 0:8 bq^T, 8:16 bk^T, 16:1040 bv broadcast
    constf_d = nc.declare_dram_parameter("constf", [128, 1040], F32, isOutput=False)
    trim_d = nc.declare_dram_parameter("trim", [128, 128], BF16, isOutput=False)
    out_d = nc.declare_dram_parameter("out", [H, 128, NS, 128], F32, isOutput=True)

    with tile.TileContext(nc) as tc:
        with (
            tc.tile_pool(name="const", bufs=1) as constp,
            tc.tile_pool(name="xt", bufs=1) as xtp,
            tc.tile_pool(name="wv", bufs=1) as wvp,
            tc.tile_pool(name="wcol", bufs=3) as wcolp,
            tc.tile_pool(name="qk", bufs=1) as qkp,
            tc.tile_pool(name="vt", bufs=1) as vp,
            tc.tile_pool(name="score", bufs=4) as scorep,
            tc.tile_pool(name="ebuf", bufs=3) as ep,
            tc.tile_pool(name="outt", bufs=2) as outp,
            tc.tile_pool(name="rcp", bufs=4) as rcpp,
            tc.tile_pool(name="psA", bufs=3, space=bass.MemorySpace.PSUM) as psA,
            tc.tile_pool(name="psS", bufs=2, space=bass.MemorySpace.PSUM) as psS,
            tc.tile_pool(name="psC", bufs=3, space=bass.MemorySpace.PSUM) as psC,
        ):
            # constants (gpsimd queue, issued first)
            constf = constp.tile([128, 1040], F32, name="constf", tag="constf")
            nc.gpsimd.dma_start(constf[:], constf_d[:])
            trim = constp.tile([128, 128], BF16, name="trim", tag="trim")
            nc.gpsimd.dma_start(trim[:], trim_d[:])
            bqt = constf[:, 0:8]
            bkt = constf[:, 8:16]
            bvb = constf[:, 16:1040]

            # PE warmup: dummy matmuls with no DMA deps bridge the DMA ramp and
            # flip the HAM clock gate to 2.4GHz before real matmuls arrive.
            wu_a = constp.tile([128, 128], BF16, name="wu_a", tag="wu_a")
            nc.vector.memset(wu_a[:], 0.0)
            wu_b = constp.tile([128, 512], BF16, name="wu_b", tag="wu_b")
            nc.vector.memset(wu_b[:], 0.0)
            for _ in range(10):
                wups = psS.tile([128, 512], F32, name="wups", tag="psS")
                nc.tensor.matmul(wups[:], wu_a[:], wu_b[:], start=True, stop=True)

            wc_of = {}

            def load_wcol(h, eng=None):
                eng = eng or nc.scalar
                wqk = wcolp.tile([128, 2, NF, 128], BF16, name="wqk", tag="wqk")
                eng.dma_start(wqk[:], wqk_d[h])
                wc_of[h] = wqk

            # first head's weights on the scalar queue; xT thirds across
            # the three DMA queues in consumption order
            load_wcol(0, nc.scalar)
            xtq = [
                xtp.tile([128, 3, S], BF16, name="xtq0", tag="xtq0"),
                xtp.tile([128, 3, S], BF16, name="xtq1", tag="xtq1"),
                xtp.tile([128, 2, S], BF16, name="xtq2", tag="xtq2"),
            ]
            nc.sync.dma_start(xtq[0][:], xt_d[:, 0:3, :])
            nc.scalar.dma_start(xtq[1][:], xt_d[:, 3:6, :])
            nc.gpsimd.dma_start(xtq[2][:], xt_d[:, 6:8, :])

            def xts(fb):
                return xtq[fb // 3][:, fb % 3, :] if fb < 6 else xtq[2][:, fb - 6, :]

            # big persistent tensors
            qT = qkp.tile([128, H, S], BF16, name="qT", tag="qT")  # [dh, head, s]
            kT = qkp.tile([128, H, S], BF16, name="kT", tag="kT")
            vv = vp.tile([128, NS, H * 129], BF16, name="vv", tag="vv")
            wv_sb = wvp.tile([128, NF, HID], BF16, name="wv", tag="wv")
            nc.gpsimd.dma_start(wv_sb[:], wv_d[:])

            # ones columns of v~ (denominator trick)
            ones_ap = vv.rearrange("p si (h d) -> p si h d", d=129)[:, :, :, 128:129]
            nc.vector.memset(ones_ap, 1.0)

            def proj_qk(h):
                # f-outer, two PSUM groups at a time (q then k) so a PSUM bank
                # stays free for ctx recycling.
                wqk = wc_of.pop(h)
                for qk, bt, dest in ((0, bqt, qT), (1, bkt, kT)):
                    pss = [
                        psA.tile([128, 512], F32, name="psA", tag="psA")
                        for _ in range(2)
                    ]
                    for fb in range(NF):
                        for c in range(2):
                            nc.tensor.matmul(
                                pss[c][:],
                                wqk[:, qk, fb, :],
                                xts(fb)[:, c * 512 : (c + 1) * 512],
                                start=(fb == 0),
                                stop=(fb == NF - 1),
                            )
                    for c in range(2):
                        nc.vector.tensor_scalar_add(
                            dest[:, h, c * 512 : (c + 1) * 512],
                            pss[c][:],
                            bt[:, h : h + 1],
                        )

            def proj_v():
                for si in range(NS):
                    for c in range(2):
                        ps = psA.tile([128, 512], F32, name="psA", tag="psA")
                        for fb in range(NF):
                            nc.tensor.matmul(
                                ps[:],
                                xts(fb)[:, si * 128 : (si + 1) * 128],
                                wv_sb[:, fb, c * 512 : (c + 1) * 512],
                                start=(fb == 0),
                                stop=(fb == NF - 1),
                            )
                        dst = vv[:, si, :].rearrange("p (h d) -> p h d", d=129)[
                            :, 4 * c : 4 * c + 4, 0:128
                        ]
                        nc.vector.tensor_add(
                            dst,
                            ps[:].rearrange("p (h d) -> p h d", d=128),
                            bvb[:, c * 512 : (c + 1) * 512].rearrange(
                                "p (h d) -> p h d", d=128
                            ),
                        )

            def eslice(E, bi, a, b):
                # packed-trapezoid slice of strip bi, local cols [a, b)
                t = E[0] if bi < 4 else E[1]
                o = EOFF[bi] - (0 if bi < 4 else EOFF[4])
                return t[:, o + a : o + b]

            def attn_scores(h, E):
                # E packed trapezoid: strip bi holds sq in [bi*128, S) at
                # columns [EOFF[bi], EOFF[bi+1])
                for bi in range(NS):
                    st = scorep.tile([128, S], F32, name="st", tag="st")
                    lo = bi * 128
                    for c0 in range(lo, S, 512):
                        n = min(512, S - c0)
                        ps = psS.tile([128, 512], F32, name="psS", tag="psS")
                        nc.tensor.matmul(
                            ps[:, 0:n],
                            kT[:, h, lo : lo + 128],
                            qT[:, h, c0 : c0 + n],
                            start=True,
                            stop=True,
                        )
                        nc.scalar.activation(
                            st[:, c0 : c0 + n], ps[:, 0:n], EXP, scale=SCALE
                        )
                    eb = eslice(E, bi, 0, S - lo)
                    nc.scalar.activation(eb, st[:, lo:S], EXP)
                    ebd = eslice(E, bi, 0, 128)
                    nc.vector.tensor_mul(ebd, ebd, trim[:])

            def ctx(h, E):
                oh = outp.tile([128, NS, 128], F32, name="oh", tag="oh")
                for bj in range(NS):
                    ps = psC.tile([128, 129], F32, name="psC", tag="psC")
                    for bi in range(bj + 1):
                        nc.tensor.matmul(
                            ps[:],
                            eslice(E, bi, (bj - bi) * 128, (bj - bi) * 128 + 128),
                            vv[:, bi, h * 129 : h * 129 + 129],
                            start=(bi == 0),
                            stop=(bi == bj),
                        )
                    rc = rcpp.tile([128, 1], F32, name="rc", tag="rc")
                    nc.vector.reciprocal(rc[:], ps[:, 128:129])
                    nc.vector.tensor_scalar_mul(oh[:, bj, :], ps[:, 0:128], rc[:])
                # head-major partition-major output, split per bj-pair so the
                # final transfer after the last normalize is small
                for j0 in range(0, NS, 2):
                    nc.sync.dma_start(out_d[h][:, j0 : j0 + 2, :], oh[:, j0 : j0 + 2, :])

            E_of = {}

            def head_front(h):
                if h + 1 < H:
                    load_wcol(h + 1)
                proj_qk(h)
                elo = ep.tile([128, EOFF[4]], BF16, name="Elo", tag="Elo")
                ehi = ep.tile([128, EOFF[NS] - EOFF[4]], BF16, name="Ehi", tag="Ehi")
                E_of[h] = (elo, ehi)
                attn_scores(h, E_of[h])

            head_front(0)
            head_front(1)
            head_front(2)
            proj_v()
            ctx(0, E_of.pop(0))
            for h in range(3, H):
                head_front(h)
                ctx(h - 2, E_of.pop(h - 2))
            ctx(H - 2, E_of.pop(H - 2))
            ctx(H - 1, E_of.pop(H - 1))

    nc.finalize()
    return nc


def _get_nc():
    global _CACHED_NC
    if _CACHED_NC is None:
        _CACHED_NC = build_nc()
    return _CACHED_NC


def _prep_shared(Wq, bq, Wk, bk, Wv, bv):
    def reorder(w):
        # [f, n] -> [h, f_in_blk(partition), f_blk, c]
        return w.reshape(NF, 128, H, 128).transpose(2, 1, 0, 3)

    wqk = np.ascontiguousarray(
        np.stack([reorder(Wq), reorder(Wk)], axis=2)
    ).astype(BF_NP)  # [H, 128, 2, NF, 128]
    wvh = np.ascontiguousarray(
        Wv.reshape(NF, 128, HID).transpose(1, 0, 2)
    ).astype(BF_NP)  # [128, NF, HID]
    constf = np.empty((128, 1040), np.float32)
    constf[:, 0:8] = bq.reshape(H, 128).T
    constf[:, 8:16] = bk.reshape(H, 128).T
    constf[:, 16:1040] = np.broadcast_to(bv, (128, HID))
    return dict(
        wqk=wqk,
        wv=wvh,
        constf=constf,
        trim=np.triu(np.ones((128, 128), np.float32)).astype(BF_NP),
    )


def _prep_xt(x):
    # [S, F] -> xT [F, S] -> [128(p), NF, S] partition-major, bf16
    return np.ascontiguousarray(x.T.reshape(NF, 128, S).transpose(1, 0, 2)).astype(
        BF_NP
    )


def _unpack_out(arr):
    # [H, 128, NS, 128] -> [S, HID]
    return np.ascontiguousarray(arr.transpose(2, 1, 0, 3).reshape(S, HID))


def kernel(queries, Wq, bq, Wk, bk, Wv, bv):
    queries = np.asarray(queries, np.float32)
    shared = _prep_shared(
        np.asarray(Wq, np.float32),
        np.asarray(bq, np.float32),
        np.asarray(Wk, np.float32),
        np.asarray(bk, np.float32),
        np.asarray(Wv, np.float32),
        np.asarray(bv, np.float32),
    )
    in_maps = [dict(xt=_prep_xt(queries[b]), **shared) for b in range(B)]
    nc = _get_nc()
    res = run_bass_kernel_spmd(nc, in_maps, core_ids=list(range(B)))
    return np.stack([_unpack_out(res.results[b]["out"]) for b in range(B)], axis=0)


if __name__ == "__main__":
    rng = np.random.default_rng(0)
    q = rng.standard_normal((B, S, FEAT), dtype=np.float32)
    Wq = (rng.standard_normal((FEAT, HID), dtype=np.float32) * 0.02).astype(np.float32)
    Wk = (rng.standard_normal((FEAT, HID), dtype=np.float32) * 0.02).astype(np.float32)
    Wv = (rng.standard_normal((FEAT, HID), dtype=np.float32) * 0.02).astype(np.float32)
    z = np.zeros(HID, np.float32)
    out = kernel(queries=q, Wq=Wq, bq=z, Wk=Wk, bk=z, Wv=Wv, bv=z)
    print(out.shape, out.dtype)
